# revision 3
# baseline (speedup 1.0000x reference)
"""DeepseekV3-style MoE block on 8 Trainium2 NeuronCores.

Strategy (expert-parallel, host-side dispatch/combine):
  - Router (sigmoid + top-2 + normalize) computed on host in fp32. Tokens
    are gathered per expert on the host (the "all-to-all dispatch") and
    core e runs expert e's SwiGLU FFN over its gathered token batch
    (padded to a common capacity, 64-token granularity).
  - Shared expert: tensor-parallel 2-way over the hidden dim (HS=1024 ->
    two 512 halves) x data-parallel 4-way over tokens. Core e computes the
    ws-half (e // 4) over token slice (e % 4).
  - Combine: host scatter-adds routed outputs (scaled by routing weights)
    and adds shared partials.

Device kernel (identical program on all 8 cores): two SwiGLU FFN passes,
each [ntok,1024] -> silu*mul -> [ntok,1024] with H=512 hidden.
Feature-major layout (features on SBUF partitions, tokens on the free
axis), so no transposes:
    hT[h,t]  = sum_d W1[d,h] * xT[d,t]      (lhsT=W1 chunk, rhs=xT chunk)
    gT[h,t]  = silu(h1T) * h3T
    y[t,d]   = sum_h gT[h,t] * W2[h,d]      (lhsT=gT chunk,  rhs=W2 chunk)

ALL device input data is packed on the host into ONE bf16 "mega panel"
[128, TOTAL] per core, laid out in exact consumption order:
    [ w13[dc] | xtr[dc] ] x8  |  w2  |  [ v13[dc] | xts[dc] ] x8  |  v2
and DMA'd as 14 large descriptors on the (in-order) DMA queue. Large
per-partition rows keep HBM read bandwidth near peak from the first
descriptor, and descriptor order matches first-use order.

Stage-1 matmuls are emitted dc-major (for dc: 4 matmuls) over hc-pairs,
so the PE's consumption rate during the initial DMA-gated window matches
descriptor arrival: the PE never idles long enough (>3.4us) for the HAM
clock gate to re-throttle it to 1.2 GHz.

PRECISION: bf16 on the wire and bf16 matmuls with fp32 PSUM accumulate;
outputs in bf16. rel err ~4e-3 (gate is 2e-2). MOE_PRECISION=f32 falls
back to fp32 wire + float32r matmuls (rel err ~3e-4).
"""

import os
import sys
from contextlib import ExitStack

import numpy as np

if "/opt/trn_rl_repo" not in sys.path and not os.path.isdir(
    os.path.join(os.path.dirname(os.path.abspath(__file__)), "concourse")
):
    sys.path.append("/opt/trn_rl_repo")

D = 1024  # model dim
E = 8  # experts
K = 2  # top-k
H = 512  # expert hidden
HS = 1024  # shared hidden
N_CORES = 8
TP_SHARED = 2  # shared expert split over HS
DP_SHARED = N_CORES // TP_SHARED  # shared expert split over tokens
KC = D // 128  # contraction chunks (dc)
HC = H // 128  # hidden chunks (hc)

PRECISION = os.environ.get("MOE_PRECISION", "bf16")
WARMUP_MM = int(os.environ.get("MOE_WARMUP", "40"))

_NC_CACHE = {}
LAST_RUN = None  # BassKernelResults of the most recent kernel() call


def _eq_groups(n, max_f=512, align=64):
    """Split n into ceil(n/max_f) near-equal multiples of `align`."""
    k = -(-n // max_f)
    gs = []
    rem = n
    for i in range(k, 0, -1):
        f = -(-rem // i)
        f = -(-f // align) * align
        f = min(f, rem)
        gs.append(f)
        rem -= f
    assert sum(gs) == n and rem == 0
    return gs


def _build_nc(cap, ts):
    """One-core Bass/Tile program: expert FFN over `cap` tokens + shared
    FFN half over `ts` tokens, reading one packed input panel.

    DRAM input (host-packed, bf16):
      xall [128, TOTAL]: per dc: w13[dc] (1024 cols = w1|w3 hidden cols)
      then xtr[dc] (cap cols); then w2 (4096 cols, hc-major); then per
      dc: v13[dc] (1024) then xts[dc] (ts/KC... 512 cols); then v2 (4096).
    Outputs: yr [cap, D] bf16 (unscaled routed), ys [ts, D] bf16.
    """
    import concourse.bacc as bacc
    import concourse.mybir as mybir
    import concourse.tile as tile

    f32 = mybir.dt.float32
    f32r = mybir.dt.float32r
    bf16 = mybir.dt.bfloat16
    AF = mybir.ActivationFunctionType

    wire = f32 if PRECISION == "f32" else bf16

    def mm(ap):
        return ap.bitcast(f32r) if PRECISION == "f32" else ap

    nc = bacc.Bacc("TRN2", target_bir_lowering=False)

    # --- panel column offsets ---
    RSEG = D + cap  # one routed dc segment: w13[dc] | xtr[dc]
    OFF_R = 0
    OFF_W2 = OFF_R + KC * RSEG
    SSEG = D + ts // 1  # shared dc segment: v13[dc] | xts[dc]
    SSEG = D + ts
    OFF_S = OFF_W2 + HC * D
    OFF_V2 = OFF_S + KC * SSEG
    TOTAL = OFF_V2 + HC * D

    xall = nc.declare_dram_parameter("xall", [128, TOTAL], wire, isOutput=False)
    yr = nc.declare_dram_parameter("yr", [cap, D], wire, isOutput=True)
    ys = nc.declare_dram_parameter("ys", [ts, D], wire, isOutput=True)

    with ExitStack() as ctx:
        tc = ctx.enter_context(tile.TileContext(nc))
        wpool = ctx.enter_context(tc.tile_pool(name="w", bufs=1))
        gpool = ctx.enter_context(tc.tile_pool(name="g", bufs=1))
        spool = ctx.enter_context(tc.tile_pool(name="s", bufs=2))
        ypool = ctx.enter_context(tc.tile_pool(name="y", bufs=2))
        hps = ctx.enter_context(tc.tile_pool(name="hps", bufs=4, space="PSUM"))
        yps = ctx.enter_context(tc.tile_pool(name="yps", bufs=3, space="PSUM"))

        mega = wpool.tile([128, TOTAL], wire, tag="mega", name="mega")

        # DMA descriptors, in consumption order. Large contiguous
        # per-partition rows -> near-peak HBM bandwidth per descriptor.
        def seg(a, b):
            nc.sync.dma_start(mm(mega[:, a:b]), mm(xall[:, a:b]))

        for dc in range(KC):  # routed: w13[dc] | xtr[dc]
            seg(OFF_R + dc * RSEG, OFF_R + (dc + 1) * RSEG)
        seg(OFF_W2, OFF_W2 + HC * D)  # w2
        for i in range(4):  # shared: (v13|xts) x2 per descriptor
            seg(OFF_S + 2 * i * SSEG, OFF_S + 2 * (i + 1) * SSEG)
        seg(OFF_V2, OFF_V2 + HC * D)  # v2

        # Warm the PE's HAM clock gate during the initial DMA wait:
        # throwaway matmuls on a zeroed tile lift the PE clock 1.2->2.4
        # GHz before the first real matmul's operands arrive.
        warm = wpool.tile([128, 128], wire, tag="warm", name="warm")
        nc.vector.memset(warm[:], 0.0)
        wp = yps.tile([128, 512], f32, tag="yp", name="wp")
        for i in range(WARMUP_MM):
            nc.tensor.matmul(
                wp[:, :128],
                mm(warm[:]),
                mm(warm[:]),
                start=(i == 0),
                stop=(i == WARMUP_MM - 1),
            )

        def ffn(x_off, xw, w13_off, w2_off, out_dram, ntok, gtag):
            """SwiGLU FFN over ntok tokens; x chunks at panel column
            x_off + dc*xw, w13 chunks at w13_off + dc*xw' ... both share
            the dc segment stride (D + xw)."""
            stride = D + xw
            g_t = [
                gpool.tile([128, xw], wire, tag=f"{gtag}{hc}", name=f"{gtag}{hc}")
                for hc in range(HC)
            ]
            # --- stage 1: hT = silu(w1T x) * (w3T x), dc-major ---
            g0 = 0
            for F in _eq_groups(ntok):
                for hp in range(2):  # hc pair
                    hcs = (2 * hp, 2 * hp + 1)
                    ht = [
                        hps.tile([128, 512], f32, tag="h", name=f"h{j}")
                        for j in range(4)
                    ]
                    for dc in range(KC):
                        wcol = w13_off + dc * stride
                        xcol = x_off + dc * stride + g0
                        for j, hc in enumerate(hcs):
                            nc.tensor.matmul(
                                ht[2 * j][:, :F],
                                mm(mega[:, wcol + hc * 128 : wcol + (hc + 1) * 128]),
                                mm(mega[:, xcol : xcol + F]),
                                start=(dc == 0),
                                stop=(dc == KC - 1),
                            )
                            nc.tensor.matmul(
                                ht[2 * j + 1][:, :F],
                                mm(
                                    mega[
                                        :,
                                        wcol + H + hc * 128 : wcol + H + (hc + 1) * 128,
                                    ]
                                ),
                                mm(mega[:, xcol : xcol + F]),
                                start=(dc == 0),
                                stop=(dc == KC - 1),
                            )
                    for j, hc in enumerate(hcs):
                        s1 = spool.tile([128, 512], f32, tag="s1", name="s1")
                        nc.scalar.activation(s1[:, :F], ht[2 * j][:, :F], AF.Silu)
                        nc.vector.tensor_mul(
                            mm(g_t[hc][:, g0 : g0 + F]),
                            s1[:, :F],
                            ht[2 * j + 1][:, :F],
                        )
                g0 += F
            # --- stage 2: y = gT.T @ w2, m-tiles of <=128 tokens ---
            for mt in range(-(-ntok // 128)):
                r0 = mt * 128
                w = min(128, ntok - r0)
                y_sb = ypool.tile([128, D], wire, tag="ysb", name="ysb")
                for nh in range(2):
                    yp = yps.tile([128, 512], f32, tag="yp", name="yp")
                    for hc in range(HC):
                        nc.tensor.matmul(
                            yp[:w, :],
                            mm(g_t[hc][:, r0 : r0 + w]),
                            mm(
                                mega[
                                    :,
                                    w2_off
                                    + hc * D
                                    + nh * 512 : w2_off
                                    + hc * D
                                    + (nh + 1) * 512,
                                ]
                            ),
                            start=(hc == 0),
                            stop=(hc == HC - 1),
                        )
                    if nh == 0:
                        nc.scalar.activation(
                            mm(y_sb[:w, 0:512]), yp[:w, :], AF.Copy
                        )
                    else:
                        nc.vector.tensor_copy(mm(y_sb[:w, 512:1024]), yp[:w, :])
                nc.sync.dma_start(mm(out_dram[r0 : r0 + w, :]), mm(y_sb[:w, :]))

        ffn(OFF_R + D, cap, OFF_R, OFF_W2, yr, cap, "gr")
        ffn(OFF_S + D, ts, OFF_S, OFF_V2, ys, ts, "gs")

    nc.compile()
    return nc


def kernel(x, gate_w, w1, w3, w2, ws1, ws3, ws2):
    global LAST_RUN
    from concourse.bass_utils import run_bass_kernel_spmd

    x = np.asarray(x, dtype=np.float32)
    gate_w = np.asarray(gate_w, dtype=np.float32)
    w1 = np.asarray(w1, dtype=np.float32)
    w3 = np.asarray(w3, dtype=np.float32)
    w2 = np.asarray(w2, dtype=np.float32)
    ws1 = np.asarray(ws1, dtype=np.float32)
    ws3 = np.asarray(ws3, dtype=np.float32)
    ws2 = np.asarray(ws2, dtype=np.float32)

    if PRECISION == "f32":
        wire_np = np.float32
    else:
        import ml_dtypes

        wire_np = ml_dtypes.bfloat16

    b, s, d = x.shape
    T = b * s
    xt = np.ascontiguousarray(x.reshape(T, d))
    ts = T // DP_SHARED  # shared-expert token slice per DP group

    # ---- Router on host (fp32, matches the jax reference's selection) ----
    logits = xt @ gate_w  # [T, E]
    with np.errstate(over="ignore"):
        scores = 1.0 / (1.0 + np.exp(-logits, dtype=np.float32))
    top2 = np.argpartition(-scores, 1, axis=1)[:, :2]  # top-2 set per token
    rows = np.arange(T)
    sel_scores = scores[rows[:, None], top2]  # [T, 2]
    norm_w = sel_scores / sel_scores.sum(axis=1, keepdims=True)

    tok_ids = []
    tok_w = []
    sel = np.zeros((T, E), dtype=bool)
    wmat = np.zeros((T, E), dtype=np.float32)
    sel[rows[:, None], top2] = True
    wmat[rows[:, None], top2] = norm_w
    for e in range(E):
        ids = np.nonzero(sel[:, e])[0]
        tok_ids.append(ids)
        tok_w.append(wmat[ids, e])

    max_ne = max(len(ids) for ids in tok_ids)
    cap = max(128, -(-max_ne // 64) * 64)

    # ---- Pack the per-core mega panels (see _build_nc layout) ----
    xtT = np.ascontiguousarray(xt.T).astype(wire_np)  # [D, T]
    w13_all = np.concatenate([w1, w3], axis=2).astype(wire_np)  # [E, D, 2H]
    ws13 = np.stack(
        [
            np.concatenate(
                [ws1[:, hf * H : (hf + 1) * H], ws3[:, hf * H : (hf + 1) * H]],
                axis=1,
            )
            for hf in range(TP_SHARED)
        ]
    ).astype(wire_np)  # [2, D, 2H]
    w2_b = w2.astype(wire_np)  # [E, H, D]
    ws2_b = ws2.astype(wire_np)  # [HS, D]

    RSEG = D + cap
    SSEG = D + ts
    TOTAL = KC * RSEG + HC * D + KC * SSEG + HC * D
    OFF_W2 = KC * RSEG
    OFF_S = OFF_W2 + HC * D
    OFF_V2 = OFF_S + KC * SSEG

    in_maps = []
    for e in range(E):
        ids = tok_ids[e]
        sl = e % DP_SHARED
        hf = e // DP_SHARED
        panel = np.zeros((128, TOTAL), dtype=wire_np)
        for dc in range(KC):
            c0 = dc * RSEG
            panel[:, c0 : c0 + D] = w13_all[e, dc * 128 : (dc + 1) * 128, :]
            panel[:, c0 + D : c0 + D + len(ids)] = xtT[
                dc * 128 : (dc + 1) * 128, ids
            ]
        for hc in range(HC):
            panel[:, OFF_W2 + hc * D : OFF_W2 + (hc + 1) * D] = w2_b[
                e, hc * 128 : (hc + 1) * 128, :
            ]
        for dc in range(KC):
            c0 = OFF_S + dc * SSEG
            panel[:, c0 : c0 + D] = ws13[hf, dc * 128 : (dc + 1) * 128, :]
            panel[:, c0 + D : c0 + D + ts] = xtT[
                dc * 128 : (dc + 1) * 128, sl * ts : (sl + 1) * ts
            ]
        for hc in range(HC):
            panel[:, OFF_V2 + hc * D : OFF_V2 + (hc + 1) * D] = ws2_b[
                hf * H + hc * 128 : hf * H + (hc + 1) * 128, :
            ]
        in_maps.append({"xall": panel})

    key = (cap, ts, PRECISION)
    nc = _NC_CACHE.get(key)
    if nc is None:
        nc = _build_nc(cap, ts)
        _NC_CACHE[key] = nc

    last_err = None
    for _attempt in range(3):
        try:
            LAST_RUN = run_bass_kernel_spmd(nc, in_maps, list(range(N_CORES)))
            break
        except Exception as err:  # transient NRT/device failures: retry
            last_err = err
    else:
        raise last_err
    results = LAST_RUN.results

    # ---- Combine on host ----
    out = np.zeros((T, d), dtype=np.float32)
    for e in range(E):
        ids = tok_ids[e]
        out[ids] += results[e]["yr"][: len(ids)].astype(np.float32) * tok_w[e][
            :, None
        ]
        sl = e % DP_SHARED
        out[sl * ts : (sl + 1) * ts] += results[e]["ys"].astype(np.float32)
    return out.reshape(b, s, d)
